# revision 8
# baseline (speedup 1.0000x reference)
"""Trainium2 Bass kernel for nn_EquiGroupSamplingC8.

Self-contained: accepts FULL inputs, shards batch dim across 8 NeuronCores,
runs a Bass/Tile kernel per core, gathers the full output.

Math restructuring (validated exactly against the reference in fp32):
  - proj_1 (o3.Linear) folded with the SO(3)-grid synthesis matrix D_IN into a
    single weight W_in[g, f, j]; per pass of 8 channels this is one K=128
    block-diagonal matmul per 128-grid-tile:  sigT = lhsT.T @ R, relu.
  - main matmul produces feat2^T[col, f*64+bt] = D_OUTW'.T @ sigT directly in
    the transposed layout needed downstream (D_OUTW' has the sqrt(2)
    normalize2mom factor folded in).
  - proj_2 and the C8 Wigner projection are folded into one weight
    B[(l,f,u*d+m), (g,i)], so traj_c8 = F.T @ B with a single accumulated
    matmul chain (F is a partition-scatter of feat2^T).
  - x_c8 = x @ C8W^T via bf16 xbar transposes of x tiles.
"""

import math
import numpy as np

LMAX = 6
F_ACT = 16
F_MID = 64
N_A, N_B, N_C = 16, 8, 16
SQRT2 = math.sqrt(2.0)
G = 2048
NB_ROWS = 7280          # sum_l 16*(2l+1)^2
NB_PAD = 7296           # 57 * 128
KT = 57
N_CORES = 8
B_LOC = 2               # batch per core
BT = 64                 # b_loc * t
XROWS = B_LOC * 32 * 128  # 8192 x-rows per core

OFFS = []
_off = 0
for _l in range(LMAX + 1):
    OFFS.append(_off)
    _off += (2 * _l + 1) ** 2
assert _off == 455

LOFFS = []
_off = 0
for _l in range(LMAX + 1):
    LOFFS.append(_off)
    _off += 16 * (2 * _l + 1) ** 2
assert _off == NB_ROWS


# ---------------------------------------------------------------------------
# Wigner-D constants (numpy, mirrors the e3nn conventions of the reference)
# ---------------------------------------------------------------------------

def _su2_generators(j):
    m = np.arange(-j, j)
    raising = np.diag(-np.sqrt(j * (j + 1) - m * (m + 1)), k=-1).astype(complex)
    m = np.arange(-j + 1, j + 1)
    lowering = np.diag(np.sqrt(j * (j + 1) - m * (m - 1)), k=1).astype(complex)
    m = np.arange(-j, j + 1)
    return np.stack([0.5 * (raising + lowering), np.diag(1j * m), -0.5j * (raising - lowering)])


def _q_real(l):
    q = np.zeros((2 * l + 1, 2 * l + 1), dtype=complex)
    for m in range(-l, 0):
        q[l + m, l + abs(m)] = 2 ** -0.5
        q[l + m, l - abs(m)] = -1j * 2 ** -0.5
    q[l, l] = 1
    for m in range(1, l + 1):
        q[l + m, l + abs(m)] = (-1) ** m * 2 ** -0.5
        q[l + m, l - abs(m)] = 1j * (-1) ** m * 2 ** -0.5
    return (-1j) ** l * q


def _so3_generators(l):
    X = _su2_generators(l)
    Q = _q_real(l)
    return np.real(np.einsum('ij,njk,kl->nil', np.conj(Q.T), X, Q))


def _expm(A):
    w, V = np.linalg.eig(A.astype(complex))
    return np.real((V * np.exp(w)) @ np.linalg.inv(V))


def _wigner_prod_grid(lmax, a, b, c):
    blocks = []
    for l in range(lmax + 1):
        X = _so3_generators(l)
        Ea = np.stack([_expm(t * X[1]) for t in a])
        Eb = np.stack([_expm(t * X[0]) for t in b])
        Ec = np.stack([_expm(t * X[1]) for t in c])
        D = np.einsum('amn,bnp,cpq->abcmq', Ea, Eb, Ec)
        blocks.append((2 * l + 1) ** 0.5 * D.reshape(len(a) * len(b) * len(c), -1))
    return np.concatenate(blocks, -1)


def _flat_wigner_pts(lmax, a, b, c):
    blocks = []
    for l in range(lmax + 1):
        X = _so3_generators(l)
        D = np.stack([_expm(ai * X[1]) @ _expm(bi * X[0]) @ _expm(ci * X[1])
                      for ai, bi, ci in zip(a, b, c)])
        blocks.append((2 * l + 1) ** 0.5 * D.reshape(len(a), -1))
    return np.concatenate(blocks, -1)


def _c8_angles():
    th = 2 * np.pi * np.arange(8) / 8
    M = np.zeros((8, 3, 3))
    M[:, 0, 0] = np.cos(th); M[:, 0, 1] = np.sin(th)
    M[:, 1, 0] = -np.sin(th); M[:, 1, 1] = np.cos(th)
    M[:, 2, 2] = 1.0
    x = M @ np.array([0.0, 1.0, 0.0])
    b = np.arccos(np.clip(x[:, 1], -1, 1))
    a = np.arctan2(x[:, 0], x[:, 2])
    def my(t):
        c, s = np.cos(t), np.sin(t)
        return np.array([[c, 0, s], [0, 1, 0], [-s, 0, c]])
    def mx(t):
        c, s = np.cos(t), np.sin(t)
        return np.array([[1, 0, 0], [0, c, -s], [0, s, c]])
    g = np.array([np.arctan2(R[0, 2], R[0, 0])
                  for R in [(my(a[i]) @ mx(b[i])).T @ M[i] for i in range(8)]])
    return a, b, g


_CONSTS = None


def _constants():
    global _CONSTS
    if _CONSTS is None:
        A = 2 * np.pi * np.arange(N_A) / N_A
        XB, WB = np.polynomial.legendre.leggauss(N_B)
        Bb = np.arccos(XB)
        C = 2 * np.pi * np.arange(N_C) / N_C
        D_IN = _wigner_prod_grid(1, A, Bb, C).astype(np.float32)              # (G, 10)
        QW = np.broadcast_to((WB / 2.0)[None, :, None] / (N_A * N_C),
                             (N_A, N_B, N_C)).reshape(-1)
        D_OUTW = (_wigner_prod_grid(LMAX, A, Bb, C) * QW[:, None]).astype(np.float32)  # (G, 455)
        C8W = _flat_wigner_pts(LMAX, *_c8_angles()).astype(np.float32)        # (8, 455)
        _CONSTS = (D_IN, D_OUTW, C8W)
    return _CONSTS


# ---------------------------------------------------------------------------
# Host-side weight folding (weights only; per-sample data is never computed on)
# ---------------------------------------------------------------------------

def _prep_weights(w1_l0, w1_l1, w2s):
    import ml_dtypes
    D_IN, D_OUTW, C8W = _constants()
    # proj_1 combined:  feat[bt,c,i] = sum_j M1[c,i,j] * traj[bt,j]
    M1 = np.zeros((F_ACT, 10, 10), np.float32)
    M1[:, 0, 9] = w1_l0[0, :, 0, 0]
    s1 = 3.0 ** -0.5
    for w_ in range(3):
        for m in range(3):
            for u in range(3):
                M1[:, 1 + w_ * 3 + m, u * 3 + m] = w1_l1[0, :, u, w_] * s1
    W_in = np.einsum("gi,cij->gcj", D_IN, M1)             # (G, 16, 10)
    # stage-1 stationary operands, zero-padded to K=128 rows
    lhsT = np.zeros((2, 128, G), np.float32)
    lhsT[0, :80] = W_in[:, 0:8, :].transpose(1, 2, 0).reshape(80, G)
    lhsT[1, :80] = W_in[:, 8:16, :].transpose(1, 2, 0).reshape(80, G)
    doutw = (D_OUTW * SQRT2).astype(np.float32)           # (G, 455)
    # proj_2 + C8 folded weight B[(l,f,u*d+m), g*8+i]
    B = np.zeros((NB_PAD, 512), np.float32)
    for l in range(LMAX + 1):
        d = 2 * l + 1
        s_l = (F_ACT * d) ** -0.5
        C8b = C8W[:, OFFS[l]:OFFS[l] + d * d].reshape(8, d, d)
        blk = np.einsum("fguw,iwm->fumgi", w2s[l], C8b) * s_l
        B[LOFFS[l]:LOFFS[l] + 16 * d * d, :] = blk.reshape(16 * d * d, 512)
    c8wt = np.zeros((512, 8), np.float32)
    c8wt[:455, :] = C8W.T
    return (lhsT[0], lhsT[1], doutw,
            B.astype(ml_dtypes.bfloat16), c8wt.astype(ml_dtypes.bfloat16))


def _scatter_pieces():
    """Maximal (src-tile, dst-tile)-aligned pieces for the feat2T -> F scatter.

    Returns list of (dst_part, dst_kt, src_part, src_ct, nrows, f)."""
    pieces = []
    for l in range(LMAX + 1):
        d2 = (2 * l + 1) ** 2
        off_l = OFFS[l]
        for f in range(16):
            dst0 = LOFFS[l] + f * d2
            a = 0
            while a < d2:
                src = off_l + a
                dst = dst0 + a
                n = d2 - a
                n = min(n, 128 - src % 128, 128 - dst % 128)
                pieces.append((dst % 128, dst // 128, src % 128, src // 128, n, f))
                a += n
    return pieces


# ---------------------------------------------------------------------------
# Device kernel (one program, SPMD across 8 cores)
# ---------------------------------------------------------------------------

def _build_program():
    import concourse.bass as bass
    import concourse.tile as tile
    from concourse import bacc, mybir

    f32 = mybir.dt.float32
    f32r = mybir.dt.float32r
    bf16 = mybir.dt.bfloat16
    Relu = mybir.ActivationFunctionType.Relu

    nc = bacc.Bacc("TRN2", target_bir_lowering=False, debug=False,
                   enable_asserts=False, num_devices=N_CORES)

    x_d = nc.dram_tensor("x_in", [XROWS, 455], f32, kind="ExternalInput").ap()
    lhsT0_d = nc.dram_tensor("lhst0", [128, G], f32r, kind="ExternalInput").ap()
    lhsT1_d = nc.dram_tensor("lhst1", [128, G], f32r, kind="ExternalInput").ap()
    r_d = nc.dram_tensor("r_blk", [128, 512], f32r, kind="ExternalInput").ap()
    doutw_d = nc.dram_tensor("doutw", [G, 455], f32r, kind="ExternalInput").ap()
    b_d = nc.dram_tensor("b_mat", [NB_PAD, 512], bf16, kind="ExternalInput").ap()
    c8_d = nc.dram_tensor("c8wt", [512, 8], bf16, kind="ExternalInput").ap()
    trajo_d = nc.dram_tensor("traj_out", [BT, 512], f32, kind="ExternalOutput").ap()
    xc8t_d = nc.dram_tensor("xc8t_out", [8, XROWS], f32, kind="ExternalOutput").ap()

    with tile.TileContext(nc) as tc:
        import contextlib
        _stack = contextlib.ExitStack()
        persist = _stack.enter_context(tc.tile_pool(name="persist", bufs=1))
        def _tctile(tc, shape, dtype, name=None):
            return persist.tile(shape, dtype, name=name, tag=name)
        # ---- persistent SBUF tensors ----
        lhsT0_s = _tctile(tc, [128, G], f32r, name="lhsT0_s")
        lhsT1_s = _tctile(tc, [128, G], f32r, name="lhsT1_s")
        r_s = _tctile(tc, [128, 512], f32r, name="r_s")
        doutw_s = _tctile(tc, [128, 16, 455], f32r, name="doutw_s")
        b_s = _tctile(tc, [128, KT, 512], bf16, name="b_s")
        c8_s = _tctile(tc, [128, 4, 8], bf16, name="c8_s")
        feat2t_s = _tctile(tc, [128, 4, 1024], bf16, name="feat2t_s")
        f_s = _tctile(tc, [128, KT, 64], bf16, name="f_s")

        nc.sync.dma_start(lhsT0_s[:], lhsT0_d)
        nc.sync.dma_start(lhsT1_s[:], lhsT1_d)
        nc.sync.dma_start(r_s[:], r_d)
        nc.sync.dma_start(doutw_s[:], doutw_d.rearrange("(gt p) c -> p gt c", p=128))
        nc.sync.dma_start(b_s[:], b_d.rearrange("(kt p) n -> p kt n", p=128))
        nc.sync.dma_start(c8_s[:], c8_d.rearrange("(t p) i -> p t i", p=128))
        # zero F pad rows (7280..7296) so pad-row garbage can't poison proj2
        nc.vector.memset(f_s[96:128, KT - 1, :], 0.0)

        with (
            tc.tile_pool(name="s1ps", bufs=2, space="PSUM") as s1_pool,
            tc.tile_pool(name="mainps", bufs=4, space="PSUM") as main_pool,
            tc.tile_pool(name="xops", bufs=2, space="PSUM") as xo_pool,
            tc.tile_pool(name="sigp", bufs=3) as sig_pool,
            tc.tile_pool(name="xtp", bufs=2) as xt_pool,
            tc.tile_pool(name="outp", bufs=3) as out_pool,
        ):
            # ================= trajectory path =================
            for pp in range(2):
                lhsT = lhsT0_s if pp == 0 else lhsT1_s
                mps = []
                for ct in range(4):
                    mp = main_pool.tile([128, 512], f32, name=f"mp{pp}_{ct}", tag="mp")
                    mps.append(mp)
                for gt in range(16):
                    s1 = s1_pool.tile([128, 512], f32, name=f"s1_{pp}_{gt}", tag="s1")
                    nc.tensor.matmul(
                        s1[:],
                        lhsT[:, gt * 128:(gt + 1) * 128],
                        r_s[:],
                        start=True, stop=True,
                    )
                    sig = sig_pool.tile([128, 512], f32r, name=f"sig{pp}_{gt}", tag="sig")
                    if gt % 2 == 0:
                        nc.vector.tensor_scalar_max(sig[:], s1[:], 0.0)
                    else:
                        nc.scalar.activation(sig[:], s1[:], Relu)
                    for ct in range(4):
                        w = 128 if ct < 3 else 71
                        nc.tensor.matmul(
                            mps[ct][:w, :],
                            doutw_s[:, gt, ct * 128:ct * 128 + w],
                            sig[:],
                            start=(gt == 0), stop=(gt == 15),
                        )
                for ct in range(4):
                    w = 128 if ct < 3 else 71
                    dst = feat2t_s[:w, ct, pp * 512:(pp + 1) * 512]
                    if ct % 2 == 0:
                        nc.vector.tensor_copy(dst, mps[ct][:w, :])
                    else:
                        nc.scalar.copy(dst, mps[ct][:w, :])

            # scatter feat2T -> F  (bf16 SBUF->SBUF partition-shuffle)
            for (dp, dkt, sp, sct, n, f) in _scatter_pieces():
                nc.sync.dma_start(
                    f_s[dp:dp + n, dkt, :],
                    feat2t_s[sp:sp + n, sct, f * 64:(f + 1) * 64],
                )

            # proj2 (+C8 folded):  out2[bt, g*8+i] = F.T @ B
            o2 = xo_pool.tile([64, 512], f32, name="o2ps", tag="xo")
            for kt in range(KT):
                nc.tensor.matmul(o2[:], f_s[:, kt, :], b_s[:, kt, :],
                                 start=(kt == 0), stop=(kt == KT - 1))
            o2_sb = out_pool.tile([64, 512], f32, name="o2sb", tag="outsb")
            nc.vector.tensor_copy(o2_sb[:], o2[:])
            nc.sync.dma_start(trajo_d, o2_sb[:])

            # ================= x path =================
            xb_tiles = []
            for i in range(6):
                xb = _tctile(tc, [128, 512], bf16, name=f"xb{i}")
                nc.vector.memset(xb[:, 455:512], 0.0)
                xb_tiles.append(xb)

            for g in range(16):
                xt = xt_pool.tile([128, 4, 128 * 4], bf16, name=f"xt{g}", tag="xt")
                for c4 in range(4):
                    chunk = g * 4 + c4
                    xb = xb_tiles[chunk % 6]
                    nc.gpsimd.dma_start(
                        xb[:, 0:455],
                        x_d[chunk * 128:(chunk + 1) * 128, :],
                    )
                    for jt in range(4):
                        nc.sync.dma_start_transpose(
                            xt[:, jt, c4 * 128:(c4 + 1) * 128],
                            xb[:, jt * 128:(jt + 1) * 128],
                        )
                xo = xo_pool.tile([8, 512], f32, name=f"xo{g}", tag="xo")
                for jt in range(4):
                    nc.tensor.matmul(xo[:], c8_s[:, jt, :], xt[:, jt, :],
                                     start=(jt == 0), stop=(jt == 3))
                xo_sb = out_pool.tile([8, 512], f32, name=f"xosb{g}", tag="outsb")
                if g % 2 == 0:
                    nc.vector.tensor_copy(xo_sb[:], xo[:])
                else:
                    nc.scalar.copy(xo_sb[:], xo[:])
                nc.sync.dma_start(xc8t_d[:, g * 512:(g + 1) * 512], xo_sb[:])

        _stack.close()

    nc.compile()
    return nc


_RUNNER = None


def _get_runner():
    """Build the program once and return a persistent callable
    run(in_maps) -> list[dict] using a cached sharded PJRT executable."""
    global _RUNNER
    if _RUNNER is not None:
        return _RUNNER

    import jax
    import numpy as np
    from jax.sharding import Mesh, PartitionSpec
    from jax.experimental.shard_map import shard_map
    from concourse import mybir
    from concourse.bass2jax import (_bass_exec_p, install_neuronx_cc_hook,
                                    partition_id_tensor)

    nc = _build_program()
    install_neuronx_cc_hook()

    pid_name = nc.partition_id_tensor.name if nc.partition_id_tensor else None
    in_names, out_names, out_avals = [], [], []
    for alloc in nc.m.functions[0].allocations:
        if not isinstance(alloc, mybir.MemoryLocationSet):
            continue
        name = alloc.memorylocations[0].name
        if alloc.kind == "ExternalInput":
            if name != pid_name:
                in_names.append(name)
        elif alloc.kind == "ExternalOutput":
            out_names.append(name)
            out_avals.append(jax.core.ShapedArray(
                tuple(alloc.tensor_shape), mybir.dt.np(alloc.dtype)))
    n_params = len(in_names)
    n_outs = len(out_avals)
    all_names = in_names + out_names
    if pid_name is not None:
        all_names = all_names + [pid_name]

    def _body(*args):
        operands = list(args)
        if pid_name is not None:
            operands.append(partition_id_tensor())
        outs = _bass_exec_p.bind(
            *operands,
            out_avals=tuple(out_avals),
            in_names=tuple(all_names),
            out_names=tuple(out_names),
            lowering_input_output_aliases=(),
            sim_require_finite=True,
            sim_require_nnan=True,
            nc=nc,
        )
        return tuple(outs)

    devices = jax.devices()[:N_CORES]
    mesh = Mesh(np.asarray(devices), ("core",))
    donate = tuple(range(n_params, n_params + n_outs))
    sharded = jax.jit(
        shard_map(_body, mesh=mesh,
                  in_specs=(PartitionSpec("core"),) * (n_params + n_outs),
                  out_specs=(PartitionSpec("core"),) * n_outs,
                  check_rep=False),
        donate_argnums=donate, keep_unused=True,
    )

    zero_shapes = [tuple(a.shape) for a in out_avals]
    zero_dtypes = [a.dtype for a in out_avals]

    def run(in_maps):
        concat_in = [
            np.concatenate([np.asarray(m[name]) for m in in_maps], axis=0)
            for name in in_names
        ]
        concat_zeros = [
            np.zeros((N_CORES * s[0],) + s[1:], dt)
            for s, dt in zip(zero_shapes, zero_dtypes)
        ]
        out_arrs = sharded(*concat_in, *concat_zeros)
        out_arrs = [np.asarray(o) for o in out_arrs]
        return [
            {name: out_arrs[i].reshape(N_CORES, *zero_shapes[i])[c]
             for i, name in enumerate(out_names)}
            for c in range(N_CORES)
        ], sharded, in_names, out_names

    _RUNNER = run
    return run


def _make_in_maps(x, trajectory, w1_l0, w1_l1, w2s):
    lhsT0, lhsT1, doutw, B, c8wt = _prep_weights(w1_l0, w1_l1, w2s)
    in_maps = []
    for c in range(N_CORES):
        traj_slice = np.ascontiguousarray(
            trajectory[c * B_LOC:(c + 1) * B_LOC].reshape(BT, 10))
        R = np.zeros((128, 512), np.float32)
        for fl in range(8):
            R[fl * 10:(fl + 1) * 10, fl * 64:(fl + 1) * 64] = traj_slice.T
        in_maps.append({
            "x_in": np.ascontiguousarray(
                x[c * B_LOC:(c + 1) * B_LOC].reshape(XROWS, 455)),
            "lhst0": lhsT0, "lhst1": lhsT1, "r_blk": R,
            "doutw": doutw, "b_mat": B, "c8wt": c8wt,
        })
    return in_maps


def kernel(x, trajectory, w1_l0, w1_l1, w2_l0, w2_l1, w2_l2, w2_l3, w2_l4,
           w2_l5, w2_l6):
    x = np.asarray(x, np.float32)
    trajectory = np.asarray(trajectory, np.float32)
    w2s = [np.asarray(w, np.float32)
           for w in (w2_l0, w2_l1, w2_l2, w2_l3, w2_l4, w2_l5, w2_l6)]
    run = _get_runner()
    in_maps = _make_in_maps(x, trajectory, np.asarray(w1_l0, np.float32),
                            np.asarray(w1_l1, np.float32), w2s)
    results, _, _, _ = run(in_maps)
    x_c8 = np.concatenate(
        [r["xc8t_out"].T.reshape(B_LOC, 32, 128, 8) for r in results], axis=0)
    traj_c8 = np.concatenate(
        [r["traj_out"].reshape(B_LOC, 32, 64, 8) for r in results], axis=0)
    return x_c8.astype(np.float32), traj_c8.astype(np.float32)
